# revision 2
# baseline (speedup 1.0000x reference)
"""Bidirectional RNN (embed -> fwd/bwd tanh scans -> vocab projection) on 8
TRN2 NeuronCores.

Strategy (per core, SPMD, identical program, no collectives):
  - Direction-split data parallelism: cores 0-3 run the FORWARD scan, cores
    4-7 the BACKWARD scan (direction chosen purely by per-core input data).
  - fc split along vocab (4 slices of 8000) x contraction (each core's
    direction contributes 512 of the 1024 rows); host sums core j + j+4.
  - HOST precompute: emb2 = (emb @ W_xh + b_h) * 64 in bf16 [VOCAB, 512].
    The embedding gather then directly yields each step's input-projection
    (x@W_xh+b), eliminating on-device staging matmuls, bias matmuls and
    PE transposes. The x64 scaling keeps W_hh representable in fp8e4m3;
    tanh applies scale=1/64 to undo it.
  - Scan: 16 fp8 (e4m3) W_hh stationaries (host-scaled x64); moving h in a
    4-slot fp8 ring. fp8 LDWEIGHTS (~27ns) hides under the 31ns matmuls.
    The step input is injected via one bf16 identity matmul from the
    DMA-xbar-transposed gather tile.
  - fc matmuls (stream-bound, 500-col) are interleaved 2 groups per scan
    step so the tensor engine never idles on the tanh round-trip; chunk c's
    steps interleave chunk c-1's fc.
  - Output written bf16 (halves DMA); host accumulates in f32.

Layouts:
  - scan z PSUM block [128, 64]: column m*16 + b for hidden row m*128+p.
  - bf16 h ring [128, 4*513*16] (col m*8208 + slot*16 + b) feeds fc as
    128-col contiguous stationary runs; fp8 h mini-ring [128, 4 slots*64].
  - xt tile [128, 4*512]: col m*512 + t holds emb2[token t][m*128+p].
"""
import numpy as np
import ml_dtypes

import concourse.bacc as bacc
import concourse.bass as bass
import concourse.mybir as mybir
import concourse.tile as tile
from concourse.bass_utils import run_bass_kernel_spmd
from concourse.masks import make_identity

P = 128
VOCAB, EMBED, HIDDEN = 32000, 256, 512
B, T = 16, 512
NCORES = 8
VSLICE = VOCAB // 4               # 8000 vocab cols per core (pairs share)
PANW = 500                        # cols per fc PSUM chunk
NCHUNK_V = VSLICE // PANW         # 16
MT = HIDDEN // P                  # 4 hidden tiles
NTOK = B * T                      # 8192
NG = NTOK // P                    # 64 gathers
CHTOK = 512                       # tokens per chunk
NCH = NTOK // CHTOK               # 16 chunks
SPC = CHTOK // B                  # 32 steps per chunk
SLOT = 16
MBLK = (T + 1) * SLOT             # 8208 h-ring cols per m-block
RS = 4                            # fp8 h ring slots
WSCALE = 64.0                     # host scaling on W_hh/emb2 (undone in tanh)
FP8_SCAN = True                   # False -> bf16 W_hh stationaries
BF = mybir.dt.bfloat16
F8 = mybir.dt.float8e4
F32 = mybir.dt.float32

_CACHED_NC = None


def build():
    nc = bacc.Bacc(None, target_bir_lowering=False, debug=False)

    emb2 = nc.declare_dram_parameter("emb2", [VOCAB, HIDDEN], BF,
                                     isOutput=False)
    ids_in = nc.declare_dram_parameter("ids_a", [P, NG], mybir.dt.int32,
                                       isOutput=False)
    wdt = F8 if FP8_SCAN else BF
    whh_in = nc.declare_dram_parameter("whh8", [HIDDEN, HIDDEN], wdt,
                                       isOutput=False)
    h0_in = nc.declare_dram_parameter("h0", [P, MT * B], F32, isOutput=False)
    wfc_in = nc.declare_dram_parameter("wfc_a", [HIDDEN, VSLICE], F32,
                                       isOutput=False)
    out = nc.declare_dram_parameter("out", [NTOK, VSLICE], BF, isOutput=True)

    from contextlib import ExitStack
    with tile.TileContext(nc) as tc:
        with tc.tile_pool(name="const", bufs=1) as const, \
             tc.tile_pool(name="hpool", bufs=1) as hpool, \
             tc.tile_pool(name="wfcp", bufs=1) as wfcp, \
             tc.tile_pool(name="evp", bufs=1) as evp, \
             tc.tile_pool(name="ps", bufs=2, space="PSUM") as ps:
            stackA = ExitStack()
            stage = stackA.enter_context(tc.tile_pool(name="stage", bufs=2))
            gat = stackA.enter_context(tc.tile_pool(name="gat", bufs=2))
            xtp = stackA.enter_context(tc.tile_pool(name="xt", bufs=2))

            # ---------------- constants ----------------
            ident_f = const.tile([P, P], F32, tag="ident_f")
            make_identity(nc, ident_f[:])
            ident_b = const.tile([P, P], BF, tag="ident_b")
            nc.vector.tensor_copy(out=ident_b[:], in_=ident_f[:])

            whh = {}
            for kt in range(MT):
                for mt in range(MT):
                    wc = const.tile([P, P], wdt, tag=f"whh{kt}{mt}", name="wc")
                    nc.sync.dma_start(
                        out=wc[:], in_=whh_in[kt * P:(kt + 1) * P,
                                              mt * P:(mt + 1) * P])
                    whh[(kt, mt)] = wc

            ids_sb = const.tile([P, NG], mybir.dt.int32, tag="ids", name="ids")
            nc.sync.dma_start(out=ids_sb[:], in_=ids_in[:, :])
            h0f = const.tile([P, MT * B], F32, tag="h0f")
            nc.sync.dma_start(out=h0f[:], in_=h0_in[:, :])

            # W_fc resident: 4 k-tiles [128, VSLICE] bf16 via f32 staging
            wfc = {}
            for kt in range(MT):
                wfb = wfcp.tile([P, VSLICE], BF, tag=f"wfc{kt}", name="wfb")
                wfc[kt] = wfb
            for kt in range(MT):
                for q in range(4):
                    qw = VSLICE // 4
                    wfs = stage.tile([P, qw], F32, tag="wfcstage", name="wfs")
                    nc.sync.dma_start(
                        out=wfs[:],
                        in_=wfc_in[kt * P:(kt + 1) * P, q * qw:(q + 1) * qw])
                    nc.vector.tensor_copy(out=wfc[kt][:, q * qw:(q + 1) * qw],
                                          in_=wfs[:])

            # h rings
            hbig = hpool.tile([P, MT * MBLK], BF, tag="hbig", name="hbig")
            hr8 = hpool.tile([P, RS * MT * B], F8, tag="hr8", name="hr8")

            def hslot_w(slot):
                return hbig[:].rearrange(
                    "p (m s) -> p m s", m=MT)[:, :, slot * SLOT:(slot + 1) * SLOT]

            nc.vector.tensor_copy(
                out=hslot_w(0), in_=h0f[:].rearrange("p (m s) -> p m s", m=MT))
            nc.vector.tensor_copy(out=hr8[:, 0:MT * B], in_=h0f[:])

            evict_flip = [0]

            def evict_engine():
                evict_flip[0] ^= 1
                return nc.vector if evict_flip[0] else nc.scalar

            # ---------------- chunk prologue: gather + xbar transpose ------
            xg_pend = {}
            xt_cur = {}

            def emit_gathers(c):
                for g in range(CHTOK // P):
                    gi = c * (CHTOK // P) + g
                    xg = gat.tile([P, HIDDEN], BF, tag=f"xg{g}", name="xg")
                    nc.gpsimd.indirect_dma_start(
                        out=xg[:], out_offset=None, in_=emb2[:],
                        in_offset=bass.IndirectOffsetOnAxis(
                            ap=ids_sb[:, gi:gi + 1], axis=0),
                    )
                    xg_pend[(c, g)] = xg

            def emit_transposes(c):
                xt = xtp.tile([P, MT * CHTOK], BF, tag="xt", name="xt")
                for g in range(CHTOK // P):
                    xg = xg_pend.pop((c, g))
                    for mt in range(MT):
                        nc.sync.dma_start(
                            out=xt[:, mt * CHTOK + g * P:
                                   mt * CHTOK + (g + 1) * P],
                            in_=xg[:, mt * P:(mt + 1) * P],
                            transpose=True)
                xt_cur[c] = xt

            # ---------------- fc group for one (token-tile, vocab-chunk) ---
            def emit_fc_group(ttile, vch):
                t0 = ttile * 8
                z = ps.tile([P, PANW], F32, tag=f"big{vch % 2}", name="z")
                for kt in range(MT):
                    lhsT = hbig[:, kt * MBLK + (t0 + 1) * SLOT:
                                kt * MBLK + (t0 + 1) * SLOT + P]
                    nc.tensor.matmul(out=z[:], lhsT=lhsT,
                                     rhs=wfc[kt][:, vch * PANW:
                                                 (vch + 1) * PANW],
                                     start=(kt == 0), stop=(kt == MT - 1))
                ev = evp.tile([P, PANW], BF, tag=f"ev{vch % 4}", name="ev")
                eng = evict_engine()
                if eng is nc.scalar:
                    nc.scalar.activation(
                        out=ev[:], in_=z[:],
                        func=mybir.ActivationFunctionType.Copy)
                else:
                    nc.vector.tensor_copy(out=ev[:], in_=z[:])
                nc.sync.dma_start(
                    out=out[ttile * P:(ttile + 1) * P,
                            vch * PANW:(vch + 1) * PANW],
                    in_=ev[:])

            # ---------------- scan step ----------------
            def emit_step(gs, xt):
                s = gs % SPC
                slot_r = gs % RS
                slot_w = (gs + 1) % RS
                z = ps.tile([P, MT * B], F32, tag=f"zscan{gs % 2}", name="z")
                # inject x-projection (emb2 gather, already scaled x64)
                rhs = xt[:].rearrange("p (m t) -> p m t", m=MT)[
                    :, :, s * B:(s + 1) * B]
                nc.tensor.matmul(out=z[:], lhsT=ident_b[:], rhs=rhs,
                                 start=True, stop=False, skip_group_check=True)
                for mt in range(MT):
                    for kt in range(MT):
                        nc.tensor.matmul(
                            out=z[:, mt * B:(mt + 1) * B],
                            lhsT=whh[(kt, mt)][:],
                            rhs=hr8[:, slot_r * MT * B + kt * B:
                                    slot_r * MT * B + (kt + 1) * B],
                            start=False,
                            stop=(mt == MT - 1 and kt == MT - 1),
                            skip_group_check=True)
                # tanh -> fp8 ring (critical path) and bf16 ring (for fc)
                nc.scalar.activation(
                    out=hr8[:, slot_w * MT * B:(slot_w + 1) * MT * B],
                    in_=z[:], func=mybir.ActivationFunctionType.Tanh,
                    scale=1.0 / WSCALE)
                nc.scalar.activation(
                    out=hslot_w(gs + 1), in_=z[:],
                    func=mybir.ActivationFunctionType.Tanh,
                    scale=1.0 / WSCALE)

            # ---------------- main loop ----------------
            emit_gathers(0)
            emit_transposes(0)
            for c in range(NCH):
                if c + 1 < NCH:
                    emit_gathers(c + 1)
                    emit_transposes(c + 1)
                fcq = []
                if c > 0:
                    fcq = [(ttile, vch)
                           for ttile in range(4 * (c - 1), 4 * c)
                           for vch in range(NCHUNK_V)]
                xt = xt_cur.pop(c)
                for s in range(SPC):
                    gs = c * SPC + s
                    if fcq:
                        emit_fc_group(*fcq.pop(0))
                    emit_step(gs, xt)
                    if fcq:
                        emit_fc_group(*fcq.pop(0))
            for ttile in range(4 * (NCH - 1), 4 * NCH):
                for vch in range(NCHUNK_V):
                    emit_fc_group(ttile, vch)

            stackA.close()
    nc.finalize()
    return nc


def _pack_h(hT):
    # [H, B] -> [128, MT*B] packed (col = m*16+b)
    return np.ascontiguousarray(
        hT.reshape(MT, P, B).transpose(1, 0, 2).reshape(P, MT * B))


def make_in_maps(inputs, h_prev, emb, W_xh_f, W_hh_f, b_h_f,
                 W_xh_b, W_hh_b, b_h_b, W_fc, b_fc):
    inputs = np.asarray(inputs, dtype=np.int32)
    ids = {"f": inputs, "b": inputs[:, ::-1]}
    emb = np.asarray(emb, np.float32)
    W_xh = {"f": np.asarray(W_xh_f, np.float32),
            "b": np.asarray(W_xh_b, np.float32)}
    W_hh = {"f": np.asarray(W_hh_f, np.float32),
            "b": np.asarray(W_hh_b, np.float32)}
    b_h = {"f": np.asarray(b_h_f, np.float32),
           "b": np.asarray(b_h_b, np.float32)}
    W_fc = np.asarray(W_fc, np.float32)
    h0 = _pack_h(np.asarray(h_prev, np.float32).T)

    wdt = ml_dtypes.float8_e4m3 if FP8_SCAN else ml_dtypes.bfloat16
    emb2 = {}
    whh8 = {}
    for d in ("f", "b"):
        emb2[d] = np.ascontiguousarray(
            ((emb @ W_xh[d]) + b_h[d]) * WSCALE).astype(ml_dtypes.bfloat16)
        whh8[d] = np.ascontiguousarray(
            (W_hh[d] * WSCALE).astype(wdt))

    in_maps = []
    for c in range(NCORES):
        d = "f" if c < 4 else "b"
        j = c % 4
        krows = slice(0, HIDDEN) if d == "f" else slice(HIDDEN, 2 * HIDDEN)
        m = {
            "emb2": emb2[d],
            "ids_a": np.ascontiguousarray(ids[d].T.reshape(NG, P).T),
            "whh8": whh8[d],
            "h0": h0,
            "wfc_a": np.ascontiguousarray(
                W_fc[krows, j * VSLICE:(j + 1) * VSLICE]),
        }
        in_maps.append(m)
    return in_maps


def assemble(results, b_fc):
    # core j (fwd) + core j+4 (bwd, time-reversed rows) sum to a vocab slice
    cols = []
    for j in range(4):
        f = results[j]["out"].astype(np.float32)
        bk = results[j + 4]["out"].astype(np.float32).reshape(
            T, B, VSLICE)[::-1].reshape(NTOK, VSLICE)
        cols.append(f + bk)
    full = np.concatenate(cols, axis=1)          # [8192, 32000], (t, b) rows
    full = full.reshape(T, B, VOCAB).transpose(1, 0, 2)
    return np.ascontiguousarray(full + np.asarray(b_fc, np.float32))


def kernel(inputs, h_prev, emb, W_xh_f, W_hh_f, b_h_f,
           W_xh_b, W_hh_b, b_h_b, W_fc, b_fc):
    global _CACHED_NC
    if _CACHED_NC is None:
        _CACHED_NC = build()
    in_maps = make_in_maps(inputs, h_prev, emb, W_xh_f, W_hh_f, b_h_f,
                           W_xh_b, W_hh_b, b_h_b, W_fc, b_fc)
    res = run_bass_kernel_spmd(_CACHED_NC, in_maps,
                               core_ids=list(range(NCORES)))
    return assemble(res.results, b_fc)


# revision 8
# speedup vs baseline: 1.2798x; 1.2798x over previous
"""Bidirectional RNN (embed -> fwd/bwd tanh scans -> vocab projection) on 8
TRN2 NeuronCores.

Strategy (per core, SPMD, identical program, no collectives):
  - Direction-split data parallelism: cores 0-3 run the FORWARD scan, cores
    4-7 the BACKWARD scan (direction chosen purely by per-core input data).
  - fc split along vocab (4 slices of 8000) x contraction (each core's
    direction contributes 512 of the 1024 rows); host sums core j + j+4.
  - HOST precompute: emb2 = (emb @ W_xh + b_h) * 64 in bf16 [VOCAB, 512].
    The embedding gather then directly yields each step's input-projection
    (x@W_xh+b), eliminating on-device staging matmuls, bias matmuls and
    PE transposes. The x64 scaling keeps W_hh representable in fp8e4m3;
    tanh applies scale=1/64 to undo it.
  - Scan: 16 fp8 (e4m3) W_hh stationaries (host-scaled x64); moving h in a
    4-slot fp8 ring. fp8 LDWEIGHTS (~27ns) hides under the 31ns matmuls.
    The step input is injected via one bf16 identity matmul from the
    DMA-xbar-transposed gather tile.
  - fc matmuls (stream-bound, 500-col) are interleaved 2 groups per scan
    step so the tensor engine never idles on the tanh round-trip; chunk c's
    steps interleave chunk c-1's fc.
  - Output written bf16 (halves DMA); host accumulates in f32.

Layouts:
  - scan z PSUM block [128, 64]: column m*16 + b for hidden row m*128+p.
  - bf16 h ring [128, 4*513*16] (col m*8208 + slot*16 + b) feeds fc as
    128-col contiguous stationary runs; fp8 h mini-ring [128, 4 slots*64].
  - xt tile [128, 4*512]: col m*512 + t holds emb2[token t][m*128+p].
"""
import numpy as np
import ml_dtypes

import concourse.bacc as bacc
import concourse.bass as bass
import concourse.mybir as mybir
import concourse.tile as tile
from concourse.bass_utils import run_bass_kernel_spmd
from concourse.masks import make_identity

P = 128
VOCAB, EMBED, HIDDEN = 32000, 256, 512
B, T = 16, 512
NCORES = 8
VSLICE = VOCAB // 4               # 8000 vocab cols per core (pairs share)
PANW = 500                        # cols per fc PSUM chunk
NCHUNK_V = VSLICE // PANW         # 16
MT = HIDDEN // P                  # 4 hidden tiles
NTOK = B * T                      # 8192
NG = NTOK // P                    # 64 gathers
CHTOK = 512                       # tokens per chunk
NCH = NTOK // CHTOK               # 16 chunks
SPC = CHTOK // B                  # 32 steps per chunk
SLOT = 16
MBLK = (T + 1) * SLOT             # 8208 h-ring cols per m-block
RS = 4                            # fp8 h ring slots
WSCALE = 64.0                     # host scaling on W_hh/emb2 (undone in tanh)
FP8_SCAN = True                   # False -> bf16 W_hh stationaries
BF = mybir.dt.bfloat16
F8 = mybir.dt.float8e4
F32 = mybir.dt.float32

_CACHED_NC = None


def build():
    nc = bacc.Bacc(None, target_bir_lowering=False, debug=False)

    emb2 = nc.declare_dram_parameter("emb2", [VOCAB, HIDDEN], BF,
                                     isOutput=False)
    ids_in = nc.declare_dram_parameter("ids_a", [P, NG], mybir.dt.int32,
                                       isOutput=False)
    wdt = F8 if FP8_SCAN else BF
    whh_in = nc.declare_dram_parameter("whh8", [HIDDEN, HIDDEN], wdt,
                                       isOutput=False)
    h0_in = nc.declare_dram_parameter("h0", [P, MT * B], F32, isOutput=False)
    wfc_in = nc.declare_dram_parameter("wfc_a", [HIDDEN, VSLICE], BF,
                                       isOutput=False)
    out = nc.declare_dram_parameter("out", [NTOK, VSLICE], BF, isOutput=True)

    from contextlib import ExitStack
    with tile.TileContext(nc) as tc:
        with tc.tile_pool(name="const", bufs=1) as const, \
             tc.tile_pool(name="hpool", bufs=1) as hpool, \
             tc.tile_pool(name="wfcp", bufs=1) as wfcp, \
             tc.tile_pool(name="evp", bufs=2) as evp, \
             tc.tile_pool(name="ps", bufs=2, space="PSUM") as ps:
            stackA = ExitStack()
            stage = stackA.enter_context(tc.tile_pool(name="stage", bufs=2))
            gat = stackA.enter_context(tc.tile_pool(name="gat", bufs=2))
            xtp = stackA.enter_context(tc.tile_pool(name="xt", bufs=2))

            # ---------------- constants ----------------
            ident_f = const.tile([P, P], F32, tag="ident_f")
            make_identity(nc, ident_f[:])
            ident_b = const.tile([P, P], BF, tag="ident_b")
            nc.vector.tensor_copy(out=ident_b[:], in_=ident_f[:])

            whh = {}
            for kt in range(MT):
                for mt in range(MT):
                    wc = const.tile([P, P], wdt, tag=f"whh{kt}{mt}", name="wc")
                    nc.sync.dma_start(
                        out=wc[:], in_=whh_in[kt * P:(kt + 1) * P,
                                              mt * P:(mt + 1) * P])
                    whh[(kt, mt)] = wc

            ids_sb = const.tile([P, NG], mybir.dt.int32, tag="ids", name="ids")
            nc.sync.dma_start(out=ids_sb[:], in_=ids_in[:, :])
            h0f = const.tile([P, MT * B], F32, tag="h0f")
            nc.sync.dma_start(out=h0f[:], in_=h0_in[:, :])

            # W_fc resident: 4 k-tiles [128, VSLICE] bf16, direct from host
            wfc = {}
            for kt in range(MT):
                wfb = wfcp.tile([P, VSLICE], BF, tag=f"wfc{kt}", name="wfb")
                nc.sync.dma_start(out=wfb[:],
                                  in_=wfc_in[kt * P:(kt + 1) * P, :])
                wfc[kt] = wfb

            # h rings
            hbig = hpool.tile([P, MT * MBLK], BF, tag="hbig", name="hbig")
            hr8 = hpool.tile([P, RS * MT * B], F8, tag="hr8", name="hr8")

            def hslot_w(slot):
                return hbig[:].rearrange(
                    "p (m s) -> p m s", m=MT)[:, :, slot * SLOT:(slot + 1) * SLOT]

            nc.vector.tensor_copy(
                out=hslot_w(0), in_=h0f[:].rearrange("p (m s) -> p m s", m=MT))
            nc.vector.tensor_copy(out=hr8[:, 0:MT * B], in_=h0f[:])

            evict_flip = [0]

            def evict_engine():
                evict_flip[0] ^= 1
                return nc.vector if evict_flip[0] else nc.scalar

            # ---------------- chunk prologue: gather + xbar transpose ------
            xg_pend = {}
            xt_cur = {}

            def emit_gathers(c):
                for g in range(CHTOK // P):
                    gi = c * (CHTOK // P) + g
                    xg = gat.tile([P, HIDDEN], BF, tag=f"xg{g}", name="xg")
                    nc.gpsimd.indirect_dma_start(
                        out=xg[:], out_offset=None, in_=emb2[:],
                        in_offset=bass.IndirectOffsetOnAxis(
                            ap=ids_sb[:, gi:gi + 1], axis=0),
                    )
                    xg_pend[(c, g)] = xg

            def alloc_xt(c):
                xt_cur[c] = xtp.tile([P, MT * CHTOK], BF, tag="xt", name="xt")

            def emit_transpose(c, g):
                # one fused xbar transpose per gather: [128 tok, 512 dim] ->
                # out[p, m, tok] (3D, m-stride CHTOK)
                xg = xg_pend.pop((c, g))
                xt = xt_cur[c]
                out3 = xt[:].rearrange("p (m t) -> p m t", m=MT)[
                    :, :, g * P:(g + 1) * P]
                nc.sync.dma_start(out=out3, in_=xg[:], transpose=True)

            # ---------------- fc group for one (token-tile, vocab-chunk) ---
            def emit_fc_group(ttile, vch):
                t0 = ttile * 8
                z = ps.tile([P, PANW], F32, tag=f"big{vch % 2}", name="z")
                for kt in range(MT):
                    lhsT = hbig[:, kt * MBLK + (t0 + 1) * SLOT:
                                kt * MBLK + (t0 + 1) * SLOT + P]
                    nc.tensor.matmul(out=z[:], lhsT=lhsT,
                                     rhs=wfc[kt][:, vch * PANW:
                                                 (vch + 1) * PANW],
                                     start=(kt == 0), stop=(kt == MT - 1))
                ev = evp.tile([P, PANW], BF, tag=f"ev{vch % 4}", name="ev")
                eng = evict_engine()
                if eng is nc.scalar:
                    nc.scalar.activation(
                        out=ev[:], in_=z[:],
                        func=mybir.ActivationFunctionType.Copy)
                else:
                    nc.vector.tensor_copy(out=ev[:], in_=z[:])
                nc.sync.dma_start(
                    out=out[ttile * P:(ttile + 1) * P,
                            vch * PANW:(vch + 1) * PANW],
                    in_=ev[:])

            # ---------------- scan step ----------------
            def emit_step(gs, xt):
                s = gs % SPC
                slot_r = gs % RS
                slot_w = (gs + 1) % RS
                z = ps.tile([P, MT * B], F32, tag=f"zscan{gs % 2}", name="z")
                # inject x-projection (emb2 gather, already scaled x64)
                rhs = xt[:].rearrange("p (m t) -> p m t", m=MT)[
                    :, :, s * B:(s + 1) * B]
                nc.tensor.matmul(out=z[:], lhsT=ident_b[:], rhs=rhs,
                                 start=True, stop=False, skip_group_check=True)
                for mt in range(MT):
                    for kt in range(MT):
                        nc.tensor.matmul(
                            out=z[:, mt * B:(mt + 1) * B],
                            lhsT=whh[(kt, mt)][:],
                            rhs=hr8[:, slot_r * MT * B + kt * B:
                                    slot_r * MT * B + (kt + 1) * B],
                            start=False,
                            stop=(mt == MT - 1 and kt == MT - 1),
                            skip_group_check=True)
                # tanh -> fp8 ring (critical path) and bf16 ring (for fc)
                nc.scalar.activation(
                    out=hr8[:, slot_w * MT * B:(slot_w + 1) * MT * B],
                    in_=z[:], func=mybir.ActivationFunctionType.Tanh,
                    scale=1.0 / WSCALE)
                nc.scalar.activation(
                    out=hslot_w(gs + 1), in_=z[:],
                    func=mybir.ActivationFunctionType.Tanh,
                    scale=1.0 / WSCALE)

            # ---------------- main loop ----------------
            emit_gathers(0)
            alloc_xt(0)
            for g in range(CHTOK // P):
                emit_transpose(0, g)
            for c in range(NCH):
                if c + 1 < NCH:
                    emit_gathers(c + 1)
                    alloc_xt(c + 1)
                fcq = []
                if c > 0:
                    fcq = [(ttile, vch)
                           for ttile in range(4 * (c - 1), 4 * c)
                           for vch in range(NCHUNK_V)]
                xt = xt_cur.pop(c)
                for s in range(SPC):
                    gs = c * SPC + s
                    if fcq:
                        emit_fc_group(*fcq.pop(0))
                    emit_step(gs, xt)
                    if fcq:
                        emit_fc_group(*fcq.pop(0))
                    # spread next chunk's xbar transposes between steps so
                    # their dispatch cost doesn't burst the sync queue
                    if c + 1 < NCH and s in (1, 9, 17, 25):
                        emit_transpose(c + 1, s // 8)
            for ttile in range(4 * (NCH - 1), 4 * NCH):
                for vch in range(NCHUNK_V):
                    emit_fc_group(ttile, vch)

            stackA.close()
    nc.finalize()
    return nc


def _pack_h(hT):
    # [H, B] -> [128, MT*B] packed (col = m*16+b)
    return np.ascontiguousarray(
        hT.reshape(MT, P, B).transpose(1, 0, 2).reshape(P, MT * B))


def make_in_maps(inputs, h_prev, emb, W_xh_f, W_hh_f, b_h_f,
                 W_xh_b, W_hh_b, b_h_b, W_fc, b_fc):
    inputs = np.asarray(inputs, dtype=np.int32)
    ids = {"f": inputs, "b": inputs[:, ::-1]}
    emb = np.asarray(emb, np.float32)
    W_xh = {"f": np.asarray(W_xh_f, np.float32),
            "b": np.asarray(W_xh_b, np.float32)}
    W_hh = {"f": np.asarray(W_hh_f, np.float32),
            "b": np.asarray(W_hh_b, np.float32)}
    b_h = {"f": np.asarray(b_h_f, np.float32),
           "b": np.asarray(b_h_b, np.float32)}
    W_fc = np.asarray(W_fc, np.float32)
    h0 = _pack_h(np.asarray(h_prev, np.float32).T)

    wdt = ml_dtypes.float8_e4m3 if FP8_SCAN else ml_dtypes.bfloat16
    emb2 = {}
    whh8 = {}
    for d in ("f", "b"):
        emb2[d] = np.ascontiguousarray(
            ((emb @ W_xh[d]) + b_h[d]) * WSCALE).astype(ml_dtypes.bfloat16)
        whh8[d] = np.ascontiguousarray(
            (W_hh[d] * WSCALE).astype(wdt))

    in_maps = []
    for c in range(NCORES):
        d = "f" if c < 4 else "b"
        j = c % 4
        krows = slice(0, HIDDEN) if d == "f" else slice(HIDDEN, 2 * HIDDEN)
        m = {
            "emb2": emb2[d],
            "ids_a": np.ascontiguousarray(ids[d].T.reshape(NG, P).T),
            "whh8": whh8[d],
            "h0": h0,
            "wfc_a": np.ascontiguousarray(
                W_fc[krows, j * VSLICE:(j + 1) * VSLICE]).astype(
                    ml_dtypes.bfloat16),
        }
        in_maps.append(m)
    return in_maps


def assemble(results, b_fc):
    # core j (fwd) + core j+4 (bwd, time-reversed rows) sum to a vocab slice
    cols = []
    for j in range(4):
        f = results[j]["out"].astype(np.float32)
        bk = results[j + 4]["out"].astype(np.float32).reshape(
            T, B, VSLICE)[::-1].reshape(NTOK, VSLICE)
        cols.append(f + bk)
    full = np.concatenate(cols, axis=1)          # [8192, 32000], (t, b) rows
    full = full.reshape(T, B, VOCAB).transpose(1, 0, 2)
    return np.ascontiguousarray(full + np.asarray(b_fc, np.float32))


def kernel(inputs, h_prev, emb, W_xh_f, W_hh_f, b_h_f,
           W_xh_b, W_hh_b, b_h_b, W_fc, b_fc):
    global _CACHED_NC
    if _CACHED_NC is None:
        _CACHED_NC = build()
    in_maps = make_in_maps(inputs, h_prev, emb, W_xh_f, W_hh_f, b_h_f,
                           W_xh_b, W_hh_b, b_h_b, W_fc, b_fc)
    res = run_bass_kernel_spmd(_CACHED_NC, in_maps,
                               core_ids=list(range(NCORES)))
    return assemble(res.results, b_fc)


# revision 11
# speedup vs baseline: 1.3434x; 1.0497x over previous
"""Bidirectional RNN (embed -> fwd/bwd tanh scans -> vocab projection) on 8
TRN2 NeuronCores.

Strategy (per core, SPMD, identical program, no collectives):
  - Direction-split data parallelism: cores 0-3 run the FORWARD scan, cores
    4-7 the BACKWARD scan (direction chosen purely by per-core input data).
  - fc split along vocab (4 slices of 8000) x contraction (each core's
    direction contributes 512 of the 1024 rows); host sums core j + j+4.
  - HOST precompute: emb2 = (emb @ W_xh + b_h) * 64 in bf16 [VOCAB, 512].
    The embedding gather then directly yields each step's input-projection
    (x@W_xh+b), eliminating on-device staging matmuls, bias matmuls and
    PE transposes. The x64 scaling keeps W_hh representable in fp8e4m3;
    tanh applies scale=1/64 to undo it.
  - Scan: 16 fp8 (e4m3) W_hh stationaries (host-scaled x64); moving h in a
    4-slot fp8 ring. fp8 LDWEIGHTS (~27ns) hides under the 31ns matmuls.
    The step input is injected via one bf16 identity matmul from the
    DMA-xbar-transposed gather tile.
  - fc matmuls (stream-bound, 500-col) are interleaved 2 groups per scan
    step so the tensor engine never idles on the tanh round-trip; chunk c's
    steps interleave chunk c-1's fc.
  - Output written bf16 (halves DMA); host accumulates in f32.

Layouts:
  - scan z PSUM block [128, 64]: column m*16 + b for hidden row m*128+p.
  - bf16 h ring [128, 4*513*16] (col m*8208 + slot*16 + b) feeds fc as
    128-col contiguous stationary runs; fp8 h mini-ring [128, 4 slots*64].
  - xt tile [128, 4*512]: col m*512 + t holds emb2[token t][m*128+p].
"""
import numpy as np
import ml_dtypes

import concourse.bacc as bacc
import concourse.bass as bass
import concourse.mybir as mybir
import concourse.tile as tile
from concourse.bass_utils import run_bass_kernel_spmd
from concourse.masks import make_identity

P = 128
VOCAB, EMBED, HIDDEN = 32000, 256, 512
B, T = 16, 512
NCORES = 8
VSLICE = VOCAB // 4               # 8000 vocab cols per core (pairs share)
PANW = 500                        # cols per fc PSUM chunk
NCHUNK_V = VSLICE // PANW         # 16
MT = HIDDEN // P                  # 4 hidden tiles
NTOK = B * T                      # 8192
NG = NTOK // P                    # 64 gathers
CHTOK = 512                       # tokens per chunk
NCH = NTOK // CHTOK               # 16 chunks
SPC = CHTOK // B                  # 32 steps per chunk
SLOT = 16
MBLK = (T + 1) * SLOT             # 8208 h-ring cols per m-block
RS = 4                            # fp8 h ring slots
WSCALE = 64.0                     # host scaling on W_hh/emb2 (undone in tanh)
FP8_SCAN = True                   # False -> bf16 W_hh stationaries
BF = mybir.dt.bfloat16
F8 = mybir.dt.float8e4
F32 = mybir.dt.float32

_CACHED_NC = None


def build():
    nc = bacc.Bacc(None, target_bir_lowering=False, debug=False)

    emb2 = nc.declare_dram_parameter("emb2", [VOCAB, HIDDEN], BF,
                                     isOutput=False)
    ids_in = nc.declare_dram_parameter("ids_a", [P, NG], mybir.dt.int32,
                                       isOutput=False)
    wdt = F8 if FP8_SCAN else BF
    whh_in = nc.declare_dram_parameter("whh8", [HIDDEN, HIDDEN], wdt,
                                       isOutput=False)
    h0_in = nc.declare_dram_parameter("h0", [P, MT * B], F32, isOutput=False)
    wfc_in = nc.declare_dram_parameter("wfc_a", [HIDDEN, VSLICE], BF,
                                       isOutput=False)
    out = nc.declare_dram_parameter("out", [NTOK, VSLICE], BF, isOutput=True)

    from contextlib import ExitStack
    with tile.TileContext(nc) as tc:
        with tc.tile_pool(name="const", bufs=1) as const, \
             tc.tile_pool(name="hpool", bufs=1) as hpool, \
             tc.tile_pool(name="wfcp", bufs=1) as wfcp, \
             tc.tile_pool(name="evp", bufs=3) as evp, \
             tc.tile_pool(name="ps", bufs=2, space="PSUM") as ps:
            stackA = ExitStack()
            stage = stackA.enter_context(tc.tile_pool(name="stage", bufs=2))
            gat = stackA.enter_context(tc.tile_pool(name="gat", bufs=2))
            xtp = stackA.enter_context(tc.tile_pool(name="xt", bufs=2))

            # ---------------- constants ----------------
            # order matters for startup latency: ids -> gathers can begin
            # immediately; weights stream in behind them.
            ids_sb = const.tile([P, NG], mybir.dt.int32, tag="ids", name="ids")
            nc.sync.dma_start(out=ids_sb[:], in_=ids_in[:, :])

            ident_f = const.tile([P, P], F32, tag="ident_f")
            make_identity(nc, ident_f[:])
            ident_b = const.tile([P, P], BF, tag="ident_b")
            nc.vector.tensor_copy(out=ident_b[:], in_=ident_f[:])
            h0f = const.tile([P, MT * B], F32, tag="h0f")
            nc.sync.dma_start(out=h0f[:], in_=h0_in[:, :])

            whh = {}
            wfc = {}

            def load_weights():
                for kt in range(MT):
                    for mt in range(MT):
                        wc = const.tile([P, P], wdt, tag=f"whh{kt}{mt}",
                                        name="wc")
                        nc.sync.dma_start(
                            out=wc[:], in_=whh_in[kt * P:(kt + 1) * P,
                                                  mt * P:(mt + 1) * P])
                        whh[(kt, mt)] = wc
                for kt in range(MT):
                    wfb = wfcp.tile([P, VSLICE], BF, tag=f"wfc{kt}",
                                    name="wfb")
                    nc.sync.dma_start(out=wfb[:],
                                      in_=wfc_in[kt * P:(kt + 1) * P, :])
                    wfc[kt] = wfb

            # h rings
            hbig = hpool.tile([P, MT * MBLK], BF, tag="hbig", name="hbig")
            hr8 = hpool.tile([P, RS * MT * B], F8, tag="hr8", name="hr8")

            def hslot_w(slot):
                return hbig[:].rearrange(
                    "p (m s) -> p m s", m=MT)[:, :, slot * SLOT:(slot + 1) * SLOT]

            nc.vector.tensor_copy(
                out=hslot_w(0), in_=h0f[:].rearrange("p (m s) -> p m s", m=MT))
            nc.vector.tensor_copy(out=hr8[:, 0:MT * B], in_=h0f[:])

            evict_flip = [0]

            def evict_engine():
                evict_flip[0] ^= 1
                return nc.vector if evict_flip[0] else nc.scalar

            # ---------------- chunk prologue: gather + xbar transpose ------
            xg_pend = {}
            xt_cur = {}

            def emit_gathers(c):
                for g in range(CHTOK // P):
                    gi = c * (CHTOK // P) + g
                    xg = gat.tile([P, HIDDEN], BF, tag=f"xg{g}", name="xg")
                    nc.gpsimd.indirect_dma_start(
                        out=xg[:], out_offset=None, in_=emb2[:],
                        in_offset=bass.IndirectOffsetOnAxis(
                            ap=ids_sb[:, gi:gi + 1], axis=0),
                    )
                    xg_pend[(c, g)] = xg

            def alloc_xt(c):
                xt_cur[c] = xtp.tile([P, MT * CHTOK], BF, tag="xt", name="xt")

            def emit_transpose(c, g):
                # one fused xbar transpose per gather: [128 tok, 512 dim] ->
                # out[p, m, tok] (3D, m-stride CHTOK)
                xg = xg_pend.pop((c, g))
                xt = xt_cur[c]
                out3 = xt[:].rearrange("p (m t) -> p m t", m=MT)[
                    :, :, g * P:(g + 1) * P]
                nc.sync.dma_start(out=out3, in_=xg[:], transpose=True)

            # ---------------- fc group for one (token-tile, vocab-chunk) ---
            def emit_fc_group(ttile, vch):
                t0 = ttile * 8
                z = ps.tile([P, PANW], F32, tag=f"big{vch % 2}", name="z")
                for kt in range(MT):
                    lhsT = hbig[:, kt * MBLK + (t0 + 1) * SLOT:
                                kt * MBLK + (t0 + 1) * SLOT + P]
                    nc.tensor.matmul(out=z[:], lhsT=lhsT,
                                     rhs=wfc[kt][:, vch * PANW:
                                                 (vch + 1) * PANW],
                                     start=(kt == 0), stop=(kt == MT - 1))
                ev = evp.tile([P, PANW], BF, tag=f"ev{vch % 4}", name="ev")
                eng = evict_engine()
                if eng is nc.scalar:
                    nc.scalar.activation(
                        out=ev[:], in_=z[:],
                        func=mybir.ActivationFunctionType.Copy)
                else:
                    nc.vector.tensor_copy(out=ev[:], in_=z[:])
                nc.sync.dma_start(
                    out=out[ttile * P:(ttile + 1) * P,
                            vch * PANW:(vch + 1) * PANW],
                    in_=ev[:])

            # ---------------- scan step ----------------
            def emit_step(gs, xt):
                s = gs % SPC
                slot_r = gs % RS
                slot_w = (gs + 1) % RS
                z = ps.tile([P, MT * B], F32, tag=f"zscan{gs % 2}", name="z")
                # inject x-projection (emb2 gather, already scaled x64)
                rhs = xt[:].rearrange("p (m t) -> p m t", m=MT)[
                    :, :, s * B:(s + 1) * B]
                nc.tensor.matmul(out=z[:], lhsT=ident_b[:], rhs=rhs,
                                 start=True, stop=False, skip_group_check=True)
                for mt in range(MT):
                    for kt in range(MT):
                        nc.tensor.matmul(
                            out=z[:, mt * B:(mt + 1) * B],
                            lhsT=whh[(kt, mt)][:],
                            rhs=hr8[:, slot_r * MT * B + kt * B:
                                    slot_r * MT * B + (kt + 1) * B],
                            start=False,
                            stop=(mt == MT - 1 and kt == MT - 1),
                            skip_group_check=True)
                # tanh -> fp8 ring (critical path) and bf16 ring (for fc)
                nc.scalar.activation(
                    out=hr8[:, slot_w * MT * B:(slot_w + 1) * MT * B],
                    in_=z[:], func=mybir.ActivationFunctionType.Tanh,
                    scale=1.0 / WSCALE)
                nc.scalar.activation(
                    out=hslot_w(gs + 1), in_=z[:],
                    func=mybir.ActivationFunctionType.Tanh,
                    scale=1.0 / WSCALE)

            # ---------------- main loop ----------------
            # fc schedule: group for token-tile T becomes safe to emit two
            # steps after T's last hidden state is produced (step T*8+7).
            fc_sched = [((tt + 1) * 8 + 2, tt, vch)
                        for tt in range(4 * NCH) for vch in range(NCHUNK_V)]
            fc_sched.reverse()          # pop() from the end

            emit_gathers(0)
            alloc_xt(0)
            for g in range(CHTOK // P):
                emit_transpose(0, g)
            load_weights()
            for c in range(NCH):
                if c + 1 < NCH:
                    emit_gathers(c + 1)
                    alloc_xt(c + 1)
                xt = xt_cur.pop(c)
                for s in range(SPC):
                    gs = c * SPC + s
                    if fc_sched and fc_sched[-1][0] <= gs:
                        emit_fc_group(*fc_sched.pop()[1:])
                    emit_step(gs, xt)
                    if fc_sched and fc_sched[-1][0] <= gs:
                        emit_fc_group(*fc_sched.pop()[1:])
                    # spread next chunk's xbar transposes between steps so
                    # their dispatch cost doesn't burst the sync queue (and
                    # lands well after their gathers complete)
                    if c + 1 < NCH and s in (3, 11, 19, 27):
                        emit_transpose(c + 1, s // 8)
            while fc_sched:
                emit_fc_group(*fc_sched.pop()[1:])

            stackA.close()
    nc.finalize()
    return nc


def _pack_h(hT):
    # [H, B] -> [128, MT*B] packed (col = m*16+b)
    return np.ascontiguousarray(
        hT.reshape(MT, P, B).transpose(1, 0, 2).reshape(P, MT * B))


def make_in_maps(inputs, h_prev, emb, W_xh_f, W_hh_f, b_h_f,
                 W_xh_b, W_hh_b, b_h_b, W_fc, b_fc):
    inputs = np.asarray(inputs, dtype=np.int32)
    ids = {"f": inputs, "b": inputs[:, ::-1]}
    emb = np.asarray(emb, np.float32)
    W_xh = {"f": np.asarray(W_xh_f, np.float32),
            "b": np.asarray(W_xh_b, np.float32)}
    W_hh = {"f": np.asarray(W_hh_f, np.float32),
            "b": np.asarray(W_hh_b, np.float32)}
    b_h = {"f": np.asarray(b_h_f, np.float32),
           "b": np.asarray(b_h_b, np.float32)}
    W_fc = np.asarray(W_fc, np.float32)
    h0 = _pack_h(np.asarray(h_prev, np.float32).T)

    wdt = ml_dtypes.float8_e4m3 if FP8_SCAN else ml_dtypes.bfloat16
    emb2 = {}
    whh8 = {}
    for d in ("f", "b"):
        emb2[d] = np.ascontiguousarray(
            ((emb @ W_xh[d]) + b_h[d]) * WSCALE).astype(ml_dtypes.bfloat16)
        whh8[d] = np.ascontiguousarray(
            (W_hh[d] * WSCALE).astype(wdt))

    in_maps = []
    for c in range(NCORES):
        d = "f" if c < 4 else "b"
        j = c % 4
        krows = slice(0, HIDDEN) if d == "f" else slice(HIDDEN, 2 * HIDDEN)
        m = {
            "emb2": emb2[d],
            "ids_a": np.ascontiguousarray(ids[d].T.reshape(NG, P).T),
            "whh8": whh8[d],
            "h0": h0,
            "wfc_a": np.ascontiguousarray(
                W_fc[krows, j * VSLICE:(j + 1) * VSLICE]).astype(
                    ml_dtypes.bfloat16),
        }
        in_maps.append(m)
    return in_maps


def assemble(results, b_fc):
    # core j (fwd) + core j+4 (bwd, time-reversed rows) sum to a vocab slice
    cols = []
    for j in range(4):
        f = results[j]["out"].astype(np.float32)
        bk = results[j + 4]["out"].astype(np.float32).reshape(
            T, B, VSLICE)[::-1].reshape(NTOK, VSLICE)
        cols.append(f + bk)
    full = np.concatenate(cols, axis=1)          # [8192, 32000], (t, b) rows
    full = full.reshape(T, B, VOCAB).transpose(1, 0, 2)
    return np.ascontiguousarray(full + np.asarray(b_fc, np.float32))


def kernel(inputs, h_prev, emb, W_xh_f, W_hh_f, b_h_f,
           W_xh_b, W_hh_b, b_h_b, W_fc, b_fc):
    global _CACHED_NC
    if _CACHED_NC is None:
        _CACHED_NC = build()
    in_maps = make_in_maps(inputs, h_prev, emb, W_xh_f, W_hh_f, b_h_f,
                           W_xh_b, W_hh_b, b_h_b, W_fc, b_fc)
    res = run_bass_kernel_spmd(_CACHED_NC, in_maps,
                               core_ids=list(range(NCORES)))
    return assemble(res.results, b_fc)


# revision 17
# speedup vs baseline: 1.3907x; 1.0352x over previous
"""Bidirectional RNN (embed -> fwd/bwd tanh scans -> vocab projection) on 8
TRN2 NeuronCores.

Strategy (per core, SPMD, identical program, no collectives):
  - Direction-split data parallelism: cores 0-3 run the FORWARD scan, cores
    4-7 the BACKWARD scan (direction chosen purely by per-core input data).
  - fc split along vocab (4 slices of 8000) x contraction (each core's
    direction contributes 512 of the 1024 rows); host sums core j + j+4.
  - HOST precompute: emb2 = (emb @ W_xh + b_h) * 64 in bf16 [VOCAB, 512].
    The embedding gather then directly yields each step's input-projection
    (x@W_xh+b), eliminating on-device staging matmuls, bias matmuls and
    PE transposes. The x64 scaling keeps W_hh representable in fp8e4m3;
    tanh applies scale=1/64 to undo it.
  - Scan: 16 fp8 (e4m3) W_hh stationaries (host-scaled x64); moving h in a
    4-slot fp8 ring. fp8 LDWEIGHTS (~27ns) hides under the 31ns matmuls.
    The step input is injected via one bf16 identity matmul from the
    DMA-xbar-transposed gather tile.
  - fc matmuls (stream-bound, 500-col) are interleaved 2 groups per scan
    step so the tensor engine never idles on the tanh round-trip; chunk c's
    steps interleave chunk c-1's fc.
  - Output written bf16 (halves DMA); host accumulates in f32.

Layouts:
  - scan z PSUM block [128, 64]: column m*16 + b for hidden row m*128+p.
  - bf16 h ring [128, 4*513*16] (col m*8208 + slot*16 + b) feeds fc as
    128-col contiguous stationary runs; fp8 h mini-ring [128, 4 slots*64].
  - xt tile [128, 4*512]: col m*512 + t holds emb2[token t][m*128+p].
"""
import numpy as np
import ml_dtypes

import concourse.bacc as bacc
import concourse.bass as bass
import concourse.mybir as mybir
import concourse.tile as tile
from concourse.bass_utils import run_bass_kernel_spmd
from concourse.masks import make_identity

P = 128
VOCAB, EMBED, HIDDEN = 32000, 256, 512
B, T = 16, 512
NCORES = 8
VSLICE = VOCAB // 4               # 8000 vocab cols per core (pairs share)
PANW = 500                        # cols per fc PSUM chunk
NCHUNK_V = VSLICE // PANW         # 16
MT = HIDDEN // P                  # 4 hidden tiles
NTOK = B * T                      # 8192
NG = NTOK // P                    # 64 gathers
CHTOK = 512                       # tokens per chunk
NCH = NTOK // CHTOK               # 16 chunks
SPC = CHTOK // B                  # 32 steps per chunk
SLOT = 16
MBLK = (T + 1) * SLOT             # 8208 h-ring cols per m-block
RS = 4                            # fp8 h ring slots
WSCALE = 64.0                     # host scaling on W_hh/emb2 (undone in tanh)
FP8_SCAN = True                   # False -> bf16 W_hh stationaries
BF = mybir.dt.bfloat16
F8 = mybir.dt.float8e4
F32 = mybir.dt.float32

_CACHED_NC = None


def build():
    nc = bacc.Bacc(None, target_bir_lowering=False, debug=False)

    emb2 = nc.declare_dram_parameter("emb2", [VOCAB, HIDDEN], BF,
                                     isOutput=False)
    ids_in = nc.declare_dram_parameter("ids_a", [P, NG], mybir.dt.int32,
                                       isOutput=False)
    wdt = F8 if FP8_SCAN else BF
    # tile-major layout: row (kt*MT+mt)*P + k_local, so each [128,128]
    # stationary is one contiguous 16KB DMA
    whh_in = nc.declare_dram_parameter("whh8", [MT * MT * P, P], wdt,
                                       isOutput=False)
    h0_in = nc.declare_dram_parameter("h0", [P, MT * B], F32, isOutput=False)
    wfc_in = nc.declare_dram_parameter("wfc_a", [HIDDEN, VSLICE], BF,
                                       isOutput=False)
    out = nc.declare_dram_parameter("out", [NTOK, VSLICE], BF, isOutput=True)

    from contextlib import ExitStack
    with tile.TileContext(nc) as tc:
        with tc.tile_pool(name="const", bufs=1) as const, \
             tc.tile_pool(name="hpool", bufs=1) as hpool, \
             tc.tile_pool(name="wfcp", bufs=1) as wfcp, \
             tc.tile_pool(name="evp", bufs=3) as evp, \
             tc.tile_pool(name="ps", bufs=2, space="PSUM") as ps:
            stackA = ExitStack()
            stage = stackA.enter_context(tc.tile_pool(name="stage", bufs=2))
            gat = stackA.enter_context(tc.tile_pool(name="gat", bufs=2))
            xtp = stackA.enter_context(tc.tile_pool(name="xt", bufs=2))

            # ---------------- constants ----------------
            # order matters for startup latency: ids -> gathers can begin
            # immediately; weights stream in behind them.
            ids_sb = const.tile([P, NG], mybir.dt.int32, tag="ids", name="ids")
            nc.sync.dma_start(out=ids_sb[:], in_=ids_in[:, :])

            ident_f = const.tile([P, P], F32, tag="ident_f")
            make_identity(nc, ident_f[:])
            ident_b = const.tile([P, P], BF, tag="ident_b")
            nc.vector.tensor_copy(out=ident_b[:], in_=ident_f[:])
            h0f = const.tile([P, MT * B], F32, tag="h0f")
            nc.sync.dma_start(out=h0f[:], in_=h0_in[:, :])

            whh = {}
            wfc = {}

            def load_whh():
                for kt in range(MT):
                    for mt in range(MT):
                        wc = const.tile([P, P], wdt, tag=f"whh{kt}{mt}",
                                        name="wc")
                        r0 = (kt * MT + mt) * P
                        nc.sync.dma_start(out=wc[:],
                                          in_=whh_in[r0:r0 + P, :])
                        whh[(kt, mt)] = wc

            def load_wfc():
                # quarter-major so the first vocab chunks of all k-tiles
                # arrive before the first fc groups need them
                qw = VSLICE // 4
                for kt in range(MT):
                    wfc[kt] = wfcp.tile([P, VSLICE], BF, tag=f"wfc{kt}",
                                        name="wfb")
                for q in range(4):
                    for kt in range(MT):
                        nc.sync.dma_start(
                            out=wfc[kt][:, q * qw:(q + 1) * qw],
                            in_=wfc_in[kt * P:(kt + 1) * P,
                                       q * qw:(q + 1) * qw])

            # h rings
            hbig = hpool.tile([P, MT * MBLK], BF, tag="hbig", name="hbig")
            hr8 = hpool.tile([P, RS * MT * B], F8, tag="hr8", name="hr8")

            def hslot_w(slot):
                return hbig[:].rearrange(
                    "p (m s) -> p m s", m=MT)[:, :, slot * SLOT:(slot + 1) * SLOT]

            nc.vector.tensor_copy(
                out=hslot_w(0), in_=h0f[:].rearrange("p (m s) -> p m s", m=MT))
            nc.vector.tensor_copy(out=hr8[:, 0:MT * B], in_=h0f[:])

            # ---------------- chunk prologue: gather + xbar transpose ------
            xg_pend = {}
            xt_cur = {}

            def emit_gathers(c):
                for g in range(CHTOK // P):
                    gi = c * (CHTOK // P) + g
                    xg = gat.tile([P, HIDDEN], BF, tag=f"xg{g}", name="xg")
                    nc.gpsimd.indirect_dma_start(
                        out=xg[:], out_offset=None, in_=emb2[:],
                        in_offset=bass.IndirectOffsetOnAxis(
                            ap=ids_sb[:, gi:gi + 1], axis=0),
                    )
                    xg_pend[(c, g)] = xg

            def alloc_xt(c):
                xt_cur[c] = xtp.tile([P, MT * CHTOK], BF, tag="xt", name="xt")

            def emit_transpose(c, g):
                # one fused xbar transpose per gather: [128 tok, 512 dim] ->
                # out[p, m, tok] (3D, m-stride CHTOK)
                xg = xg_pend.pop((c, g))
                xt = xt_cur[c]
                out3 = xt[:].rearrange("p (m t) -> p m t", m=MT)[
                    :, :, g * P:(g + 1) * P]
                nc.sync.dma_start(out=out3, in_=xg[:], transpose=True)

            # ---------------- fc group for one (token-tile, vocab-chunk) ---
            def emit_fc_group(ttile, vch):
                t0 = ttile * 8
                z = ps.tile([P, PANW], F32, tag=f"big{vch % 2}", name="z")
                for kt in range(MT):
                    lhsT = hbig[:, kt * MBLK + (t0 + 1) * SLOT:
                                kt * MBLK + (t0 + 1) * SLOT + P]
                    nc.tensor.matmul(out=z[:], lhsT=lhsT,
                                     rhs=wfc[kt][:, vch * PANW:
                                                 (vch + 1) * PANW],
                                     start=(kt == 0), stop=(kt == MT - 1))
                ev = evp.tile([P, PANW], BF, tag=f"ev{vch % 4}", name="ev")
                # evictions live on vector only: scalar runs just the tanh
                # pair, so the scan's critical tanh never queues behind an
                # eviction
                nc.vector.tensor_copy(out=ev[:], in_=z[:])
                nc.sync.dma_start(
                    out=out[ttile * P:(ttile + 1) * P,
                            vch * PANW:(vch + 1) * PANW],
                    in_=ev[:])

            # ---------------- scan step ----------------
            def emit_step(gs, xt):
                s = gs % SPC
                slot_r = gs % RS
                slot_w = (gs + 1) % RS
                z = ps.tile([P, MT * B], F32, tag=f"zscan{gs % 2}", name="z")
                # inject x-projection (emb2 gather, already scaled x64)
                rhs = xt[:].rearrange("p (m t) -> p m t", m=MT)[
                    :, :, s * B:(s + 1) * B]
                nc.tensor.matmul(out=z[:], lhsT=ident_b[:], rhs=rhs,
                                 start=True, stop=False, skip_group_check=True)
                for mt in range(MT):
                    for kt in range(MT):
                        nc.tensor.matmul(
                            out=z[:, mt * B:(mt + 1) * B],
                            lhsT=whh[(kt, mt)][:],
                            rhs=hr8[:, slot_r * MT * B + kt * B:
                                    slot_r * MT * B + (kt + 1) * B],
                            start=False,
                            stop=(mt == MT - 1 and kt == MT - 1),
                            skip_group_check=True)
                # tanh -> fp8 ring (critical path) and bf16 ring (for fc)
                nc.scalar.activation(
                    out=hr8[:, slot_w * MT * B:(slot_w + 1) * MT * B],
                    in_=z[:], func=mybir.ActivationFunctionType.Tanh,
                    scale=1.0 / WSCALE)
                nc.scalar.activation(
                    out=hslot_w(gs + 1), in_=z[:],
                    func=mybir.ActivationFunctionType.Tanh,
                    scale=1.0 / WSCALE)

            # ---------------- main loop ----------------
            # fc schedule: group for token-tile T becomes safe to emit two
            # steps after T's last hidden state is produced (step T*8+7).
            fc_sched = [((tt + 1) * 8 + 2, tt, vch)
                        for tt in range(4 * NCH) for vch in range(NCHUNK_V)]
            fc_sched.reverse()          # pop() from the end

            emit_gathers(0)
            alloc_xt(0)
            load_whh()
            for g in range(CHTOK // P):
                emit_transpose(0, g)
            load_wfc()
            for c in range(NCH):
                if c + 1 < NCH:
                    emit_gathers(c + 1)
                    alloc_xt(c + 1)
                xt = xt_cur.pop(c)
                for s in range(SPC):
                    gs = c * SPC + s
                    if fc_sched and fc_sched[-1][0] <= gs:
                        emit_fc_group(*fc_sched.pop()[1:])
                    emit_step(gs, xt)
                    if fc_sched and fc_sched[-1][0] <= gs:
                        emit_fc_group(*fc_sched.pop()[1:])
                    # spread next chunk's xbar transposes between steps so
                    # their dispatch cost doesn't burst the sync queue (and
                    # lands well after their gathers complete)
                    if c + 1 < NCH and s in (3, 11, 19, 27):
                        emit_transpose(c + 1, s // 8)
            while fc_sched:
                emit_fc_group(*fc_sched.pop()[1:])

            stackA.close()
    nc.finalize()
    return nc


def _pack_h(hT):
    # [H, B] -> [128, MT*B] packed (col = m*16+b)
    return np.ascontiguousarray(
        hT.reshape(MT, P, B).transpose(1, 0, 2).reshape(P, MT * B))


def make_in_maps(inputs, h_prev, emb, W_xh_f, W_hh_f, b_h_f,
                 W_xh_b, W_hh_b, b_h_b, W_fc, b_fc):
    inputs = np.asarray(inputs, dtype=np.int32)
    ids = {"f": inputs, "b": inputs[:, ::-1]}
    emb = np.asarray(emb, np.float32)
    W_xh = {"f": np.asarray(W_xh_f, np.float32),
            "b": np.asarray(W_xh_b, np.float32)}
    W_hh = {"f": np.asarray(W_hh_f, np.float32),
            "b": np.asarray(W_hh_b, np.float32)}
    b_h = {"f": np.asarray(b_h_f, np.float32),
           "b": np.asarray(b_h_b, np.float32)}
    W_fc = np.asarray(W_fc, np.float32)
    h0 = _pack_h(np.asarray(h_prev, np.float32).T)

    wdt = ml_dtypes.float8_e4m3 if FP8_SCAN else ml_dtypes.bfloat16
    emb2 = {}
    whh8 = {}
    for d in ("f", "b"):
        emb2[d] = np.ascontiguousarray(
            ((emb @ W_xh[d]) + b_h[d]) * WSCALE).astype(ml_dtypes.bfloat16)
        # tile-major [kt, mt, 128, 128] -> [(kt*4+mt)*128 + k_local, 128]
        w = (W_hh[d] * WSCALE).reshape(MT, P, MT, P).transpose(0, 2, 1, 3)
        whh8[d] = np.ascontiguousarray(
            w.reshape(MT * MT * P, P).astype(wdt))

    in_maps = []
    for c in range(NCORES):
        d = "f" if c < 4 else "b"
        j = c % 4
        krows = slice(0, HIDDEN) if d == "f" else slice(HIDDEN, 2 * HIDDEN)
        m = {
            "emb2": emb2[d],
            "ids_a": np.ascontiguousarray(ids[d].T.reshape(NG, P).T),
            "whh8": whh8[d],
            "h0": h0,
            "wfc_a": np.ascontiguousarray(
                W_fc[krows, j * VSLICE:(j + 1) * VSLICE]).astype(
                    ml_dtypes.bfloat16),
        }
        in_maps.append(m)
    return in_maps


def assemble(results, b_fc):
    # core j (fwd) + core j+4 (bwd, time-reversed rows) sum to a vocab slice
    cols = []
    for j in range(4):
        f = results[j]["out"].astype(np.float32)
        bk = results[j + 4]["out"].astype(np.float32).reshape(
            T, B, VSLICE)[::-1].reshape(NTOK, VSLICE)
        cols.append(f + bk)
    full = np.concatenate(cols, axis=1)          # [8192, 32000], (t, b) rows
    full = full.reshape(T, B, VOCAB).transpose(1, 0, 2)
    return np.ascontiguousarray(full + np.asarray(b_fc, np.float32))


def kernel(inputs, h_prev, emb, W_xh_f, W_hh_f, b_h_f,
           W_xh_b, W_hh_b, b_h_b, W_fc, b_fc):
    global _CACHED_NC
    if _CACHED_NC is None:
        _CACHED_NC = build()
    in_maps = make_in_maps(inputs, h_prev, emb, W_xh_f, W_hh_f, b_h_f,
                           W_xh_b, W_hh_b, b_h_b, W_fc, b_fc)
    res = run_bass_kernel_spmd(_CACHED_NC, in_maps,
                               core_ids=list(range(NCORES)))
    return assemble(res.results, b_fc)


# revision 22
# speedup vs baseline: 1.3935x; 1.0020x over previous
"""Bidirectional RNN (embed -> fwd/bwd tanh scans -> vocab projection) on 8
TRN2 NeuronCores.

Strategy (per core, SPMD, identical program, no collectives):
  - Direction-split data parallelism: cores 0-3 run the FORWARD scan, cores
    4-7 the BACKWARD scan (direction chosen purely by per-core input data).
  - fc split along vocab (4 slices of 8000) x contraction (each core's
    direction contributes 512 of the 1024 rows); host sums core j + j+4.
  - HOST precompute: emb2 = (emb @ W_xh + b_h) * 64 in bf16 [VOCAB, 512].
    The embedding gather then directly yields each step's input-projection
    (x@W_xh+b), eliminating on-device staging matmuls, bias matmuls and
    PE transposes. The x64 scaling keeps W_hh representable in fp8e4m3;
    tanh applies scale=1/64 to undo it.
  - Scan: 16 fp8 (e4m3) W_hh stationaries (host-scaled x64); moving h in a
    4-slot fp8 ring. fp8 LDWEIGHTS (~27ns) hides under the 31ns matmuls.
    The step input is injected via one bf16 identity matmul from the
    DMA-xbar-transposed gather tile.
  - fc matmuls (stream-bound, 500-col) are interleaved 2 groups per scan
    step so the tensor engine never idles on the tanh round-trip; chunk c's
    steps interleave chunk c-1's fc.
  - Output written bf16 (halves DMA); host accumulates in f32.

Layouts:
  - scan z PSUM block [128, 64]: column m*16 + b for hidden row m*128+p.
  - bf16 h ring [128, 4*513*16] (col m*8208 + slot*16 + b) feeds fc as
    128-col contiguous stationary runs; fp8 h mini-ring [128, 4 slots*64].
  - xt tile [128, 4*512]: col m*512 + t holds emb2[token t][m*128+p].
"""
import numpy as np
import ml_dtypes

import concourse.bacc as bacc
import concourse.bass as bass
import concourse.mybir as mybir
import concourse.tile as tile
from concourse.bass_utils import run_bass_kernel_spmd
from concourse.masks import make_identity

P = 128
VOCAB, EMBED, HIDDEN = 32000, 256, 512
B, T = 16, 512
NCORES = 8
VSLICE = VOCAB // 4               # 8000 vocab cols per core (pairs share)
PANW = 500                        # cols per fc PSUM chunk
NCHUNK_V = VSLICE // PANW         # 16
MT = HIDDEN // P                  # 4 hidden tiles
NTOK = B * T                      # 8192
NG = NTOK // P                    # 64 gathers
CHTOK = 512                       # tokens per chunk
NCH = NTOK // CHTOK               # 16 chunks
SPC = CHTOK // B                  # 32 steps per chunk
SLOT = 16
MBLK = (T + 1) * SLOT             # 8208 h-ring cols per m-block
RS = 4                            # fp8 h ring slots
WSCALE = 64.0                     # host scaling on W_hh/emb2 (undone in tanh)
FP8_SCAN = True                   # False -> bf16 W_hh stationaries
BF = mybir.dt.bfloat16
F8 = mybir.dt.float8e4
F32 = mybir.dt.float32

_CACHED_NC = None


def build():
    nc = bacc.Bacc(None, target_bir_lowering=False, debug=False)

    emb2 = nc.declare_dram_parameter("emb2", [VOCAB, HIDDEN], BF,
                                     isOutput=False)
    ids_in = nc.declare_dram_parameter("ids_a", [P, NG], mybir.dt.int32,
                                       isOutput=False)
    wdt = F8 if FP8_SCAN else BF
    # tile-major layout: row (kt*MT+mt)*P + k_local, so each [128,128]
    # stationary is one contiguous 16KB DMA
    whh_in = nc.declare_dram_parameter("whh8", [MT * MT * P, P], wdt,
                                       isOutput=False)
    h0_in = nc.declare_dram_parameter("h0", [P, MT * B], F32, isOutput=False)
    wfc_in = nc.declare_dram_parameter("wfc_a", [HIDDEN, VSLICE], BF,
                                       isOutput=False)
    out = nc.declare_dram_parameter("out", [NTOK, VSLICE], BF, isOutput=True)

    from contextlib import ExitStack
    with tile.TileContext(nc) as tc:
        with tc.tile_pool(name="const", bufs=1) as const, \
             tc.tile_pool(name="hpool", bufs=1) as hpool, \
             tc.tile_pool(name="wfcp", bufs=1) as wfcp, \
             tc.tile_pool(name="evp", bufs=3) as evp, \
             tc.tile_pool(name="ps", bufs=3, space="PSUM") as ps, \
             tc.tile_pool(name="psz", bufs=1, space="PSUM") as psz:
            stackA = ExitStack()
            stage = stackA.enter_context(tc.tile_pool(name="stage", bufs=2))
            gat = stackA.enter_context(tc.tile_pool(name="gat", bufs=2))
            xtp = stackA.enter_context(tc.tile_pool(name="xt", bufs=2))

            # ---------------- constants ----------------
            # order matters for startup latency: ids -> gathers can begin
            # immediately; weights stream in behind them.
            ids_sb = const.tile([P, NG], mybir.dt.int32, tag="ids", name="ids")
            nc.sync.dma_start(out=ids_sb[:], in_=ids_in[:, :])

            ident_f = const.tile([P, P], F32, tag="ident_f")
            make_identity(nc, ident_f[:])
            ident_b = const.tile([P, P], BF, tag="ident_b")
            nc.vector.tensor_copy(out=ident_b[:], in_=ident_f[:])
            h0f = const.tile([P, MT * B], F32, tag="h0f")
            nc.sync.dma_start(out=h0f[:], in_=h0_in[:, :])

            whh = {}
            wfc = {}

            def load_whh():
                for kt in range(MT):
                    for mt in range(MT):
                        wc = const.tile([P, P], wdt, tag=f"whh{kt}{mt}",
                                        name="wc")
                        r0 = (kt * MT + mt) * P
                        nc.sync.dma_start(out=wc[:],
                                          in_=whh_in[r0:r0 + P, :])
                        whh[(kt, mt)] = wc

            def load_wfc():
                # quarter-major so the first vocab chunks of all k-tiles
                # arrive before the first fc groups need them
                qw = VSLICE // 4
                for kt in range(MT):
                    wfc[kt] = wfcp.tile([P, VSLICE], BF, tag=f"wfc{kt}",
                                        name="wfb")
                for q in range(4):
                    for kt in range(MT):
                        nc.sync.dma_start(
                            out=wfc[kt][:, q * qw:(q + 1) * qw],
                            in_=wfc_in[kt * P:(kt + 1) * P,
                                       q * qw:(q + 1) * qw])

            # h rings
            hbig = hpool.tile([P, MT * MBLK], BF, tag="hbig", name="hbig")
            hr8 = hpool.tile([P, RS * MT * B], F8, tag="hr8", name="hr8")

            def hslot_w(slot):
                return hbig[:].rearrange(
                    "p (m s) -> p m s", m=MT)[:, :, slot * SLOT:(slot + 1) * SLOT]

            nc.vector.tensor_copy(
                out=hslot_w(0), in_=h0f[:].rearrange("p (m s) -> p m s", m=MT))
            nc.vector.tensor_copy(out=hr8[:, 0:MT * B], in_=h0f[:])

            # ---------------- chunk prologue: gather + xbar transpose ------
            xg_pend = {}
            xt_cur = {}

            def emit_gathers(c):
                for g in range(CHTOK // P):
                    gi = c * (CHTOK // P) + g
                    xg = gat.tile([P, HIDDEN], BF, tag=f"xg{g}", name="xg")
                    nc.gpsimd.indirect_dma_start(
                        out=xg[:], out_offset=None, in_=emb2[:],
                        in_offset=bass.IndirectOffsetOnAxis(
                            ap=ids_sb[:, gi:gi + 1], axis=0),
                    )
                    xg_pend[(c, g)] = xg

            def emit_transpose(c, g):
                # one fused xbar transpose per gather: [128 tok, 512 dim] ->
                # quarter tile [p, m, tok] (3D, m-stride P). Quarter tiles
                # keep the dependency fine-grained: step s only waits on
                # quarter s//8.
                xg = xg_pend.pop((c, g))
                xt = xtp.tile([P, MT * P], BF, tag=f"xtq{g}", name="xtq")
                out3 = xt[:].rearrange("p (m t) -> p m t", m=MT)
                nc.sync.dma_start(out=out3, in_=xg[:], transpose=True)
                xt_cur[(c, g)] = xt

            # ---------------- fc group for one (token-tile, vocab-chunk) ---
            def emit_fc_group(ttile, vch):
                t0 = ttile * 8
                z = ps.tile([P, PANW], F32, tag=f"big{vch % 2}", name="z")
                for kt in range(MT):
                    lhsT = hbig[:, kt * MBLK + (t0 + 1) * SLOT:
                                kt * MBLK + (t0 + 1) * SLOT + P]
                    nc.tensor.matmul(out=z[:], lhsT=lhsT,
                                     rhs=wfc[kt][:, vch * PANW:
                                                 (vch + 1) * PANW],
                                     start=(kt == 0), stop=(kt == MT - 1))
                ev = evp.tile([P, PANW], BF, tag=f"ev{vch % 4}", name="ev")
                # evictions live on vector only: scalar runs just the tanh
                # pair, so the scan's critical tanh never queues behind an
                # eviction
                nc.vector.tensor_copy(out=ev[:], in_=z[:])
                nc.sync.dma_start(
                    out=out[ttile * P:(ttile + 1) * P,
                            vch * PANW:(vch + 1) * PANW],
                    in_=ev[:])

            # ---------------- scan step ----------------
            def emit_step(gs):
                s = gs % SPC
                slot_r = gs % RS
                slot_w = (gs + 1) % RS
                z = psz.tile([P, MT * B], F32, tag=f"zscan{gs % 2}", name="z")
                # inject x-projection (emb2 gather, already scaled x64)
                xt = xt_cur[(gs // SPC, s // 8)]
                rhs = xt[:].rearrange("p (m t) -> p m t", m=MT)[
                    :, :, (s % 8) * B:(s % 8 + 1) * B]
                nc.tensor.matmul(out=z[:], lhsT=ident_b[:], rhs=rhs,
                                 start=True, stop=False, skip_group_check=True)
                for mt in range(MT):
                    for kt in range(MT):
                        nc.tensor.matmul(
                            out=z[:, mt * B:(mt + 1) * B],
                            lhsT=whh[(kt, mt)][:],
                            rhs=hr8[:, slot_r * MT * B + kt * B:
                                    slot_r * MT * B + (kt + 1) * B],
                            start=False,
                            stop=(mt == MT - 1 and kt == MT - 1),
                            skip_group_check=True)
                # tanh -> fp8 ring (critical path) and bf16 ring (for fc)
                nc.scalar.activation(
                    out=hr8[:, slot_w * MT * B:(slot_w + 1) * MT * B],
                    in_=z[:], func=mybir.ActivationFunctionType.Tanh,
                    scale=1.0 / WSCALE)
                nc.scalar.activation(
                    out=hslot_w(gs + 1), in_=z[:],
                    func=mybir.ActivationFunctionType.Tanh,
                    scale=1.0 / WSCALE)

            # ---------------- main loop ----------------
            # fc schedule: group for token-tile T becomes safe to emit two
            # steps after T's last hidden state is produced (step T*8+7).
            fc_sched = [((tt + 1) * 8 + 2, tt, vch)
                        for tt in range(4 * NCH) for vch in range(NCHUNK_V)]
            fc_sched.reverse()          # pop() from the end

            emit_gathers(0)
            load_whh()
            for g in range(CHTOK // P):
                emit_transpose(0, g)
            load_wfc()
            for c in range(NCH):
                if c + 1 < NCH:
                    emit_gathers(c + 1)
                for s in range(SPC):
                    gs = c * SPC + s
                    if fc_sched and fc_sched[-1][0] <= gs:
                        emit_fc_group(*fc_sched.pop()[1:])
                    emit_step(gs)
                    if fc_sched and fc_sched[-1][0] <= gs:
                        emit_fc_group(*fc_sched.pop()[1:])
                    # spread next chunk's xbar transposes between steps so
                    # their dispatch cost doesn't burst the sync queue (and
                    # lands well after their gathers complete)
                    if c + 1 < NCH and s in (3, 11, 19, 27):
                        emit_transpose(c + 1, s // 8)
                for g in range(CHTOK // P):
                    xt_cur.pop((c, g))
            while fc_sched:
                emit_fc_group(*fc_sched.pop()[1:])

            stackA.close()
    nc.finalize()
    return nc


def _pack_h(hT):
    # [H, B] -> [128, MT*B] packed (col = m*16+b)
    return np.ascontiguousarray(
        hT.reshape(MT, P, B).transpose(1, 0, 2).reshape(P, MT * B))


def make_in_maps(inputs, h_prev, emb, W_xh_f, W_hh_f, b_h_f,
                 W_xh_b, W_hh_b, b_h_b, W_fc, b_fc):
    inputs = np.asarray(inputs, dtype=np.int32)
    ids = {"f": inputs, "b": inputs[:, ::-1]}
    emb = np.asarray(emb, np.float32)
    W_xh = {"f": np.asarray(W_xh_f, np.float32),
            "b": np.asarray(W_xh_b, np.float32)}
    W_hh = {"f": np.asarray(W_hh_f, np.float32),
            "b": np.asarray(W_hh_b, np.float32)}
    b_h = {"f": np.asarray(b_h_f, np.float32),
           "b": np.asarray(b_h_b, np.float32)}
    W_fc = np.asarray(W_fc, np.float32)
    h0 = _pack_h(np.asarray(h_prev, np.float32).T)

    wdt = ml_dtypes.float8_e4m3 if FP8_SCAN else ml_dtypes.bfloat16
    emb2 = {}
    whh8 = {}
    for d in ("f", "b"):
        emb2[d] = np.ascontiguousarray(
            ((emb @ W_xh[d]) + b_h[d]) * WSCALE).astype(ml_dtypes.bfloat16)
        # tile-major [kt, mt, 128, 128] -> [(kt*4+mt)*128 + k_local, 128]
        w = (W_hh[d] * WSCALE).reshape(MT, P, MT, P).transpose(0, 2, 1, 3)
        whh8[d] = np.ascontiguousarray(
            w.reshape(MT * MT * P, P).astype(wdt))

    in_maps = []
    for c in range(NCORES):
        d = "f" if c < 4 else "b"
        j = c % 4
        krows = slice(0, HIDDEN) if d == "f" else slice(HIDDEN, 2 * HIDDEN)
        m = {
            "emb2": emb2[d],
            "ids_a": np.ascontiguousarray(ids[d].T.reshape(NG, P).T),
            "whh8": whh8[d],
            "h0": h0,
            "wfc_a": np.ascontiguousarray(
                W_fc[krows, j * VSLICE:(j + 1) * VSLICE]).astype(
                    ml_dtypes.bfloat16),
        }
        in_maps.append(m)
    return in_maps


def assemble(results, b_fc):
    # core j (fwd) + core j+4 (bwd, time-reversed rows) sum to a vocab slice
    cols = []
    for j in range(4):
        f = results[j]["out"].astype(np.float32)
        bk = results[j + 4]["out"].astype(np.float32).reshape(
            T, B, VSLICE)[::-1].reshape(NTOK, VSLICE)
        cols.append(f + bk)
    full = np.concatenate(cols, axis=1)          # [8192, 32000], (t, b) rows
    full = full.reshape(T, B, VOCAB).transpose(1, 0, 2)
    return np.ascontiguousarray(full + np.asarray(b_fc, np.float32))


def kernel(inputs, h_prev, emb, W_xh_f, W_hh_f, b_h_f,
           W_xh_b, W_hh_b, b_h_b, W_fc, b_fc):
    global _CACHED_NC
    if _CACHED_NC is None:
        _CACHED_NC = build()
    in_maps = make_in_maps(inputs, h_prev, emb, W_xh_f, W_hh_f, b_h_f,
                           W_xh_b, W_hh_b, b_h_b, W_fc, b_fc)
    res = run_bass_kernel_spmd(_CACHED_NC, in_maps,
                               core_ids=list(range(NCORES)))
    return assemble(res.results, b_fc)
